# revision 16
# baseline (speedup 1.0000x reference)
"""Trainium2 Bass kernel for the CdfgReader GNN message-passing problem.

Reference computation (shapes hardcoded):
    G, N, F, H, B, L = 4, 1024, 256, 256, 32, 4
    X = batch_xs[graph_idx]          # [B, N, F]
    A = batch_as[graph_idx]          # [B, N, N]
    x = relu(X @ w_in + b_in)
    res = x
    for i in range(L-1): x = relu(A @ x @ gcn_w[i] + gcn_b[i])
    x = tanh(A @ x @ gcn_w[L-1] + gcn_b[L-1])
    x = x + res
    out[b] = masked_mean_over_nodes(x[b], cp_mask[b])   # [B, H]

Key structural insight: the whole forward up to the final masked mean depends
only on which of the G=4 distinct graphs an example selects — so we compute
the forward once per distinct graph (4 graphs) instead of once per example
(32 examples), an 8x FLOP reduction. The per-example masked mean then becomes
a tiny [B,N]x[N,H] matmul against a host-built selection matrix.

Sharding: graph-parallel — core g (g in 0..3) computes graph g's full forward
plus its [B,H] partial of the output; cores 4..7 run the same program on
zeros. The host sums the (disjoint) partials and divides by the mask counts.

Per-core device program (measured max-rel-err ~1.6e-3 end to end — the
output is dominated by the bf16 residual path, so the whole GCN stack runs
in fp8e4m3 with per-tensor pre-scales and DoubleRow matmuls, 2 MACs/cell):
    x0  = relu(XT.T @ w_in)            bf16, 16 matmuls   (lhsT = XT)
    per layer (fp8 DoubleRow):
        zT = (x.T @ AT)                16 matmuls, contraction 256/matmul
        x' = act(zT.T @ W_l)           8 matmuls
    out_partial = R.T@x4 + R.T@x0      bf16, 16 matmuls into one PSUM
The alternating lhsT choice (x -> zT -> x) makes the chain transpose-free;
all activation/copy stages alternate between ACT and DVE so neither gates
the PE stream.

Perf notes (v1):
  - The PE runs at 1.2GHz until it has been continuously busy for ~3us, then
    2.4GHz. KWARM dummy matmuls on memset tiles run during the DMA wait so
    the real matmul stream starts at full clock.
  - xt is split into 4 quarter-descriptors on two DMA rings so the first
    input-layer m-tiles land ~1.5us earlier; each ring's first transfer pays
    ~1.5-2.4us of ring-bootstrap latency, so first-needed tensors go first.
  - Stage-A layer-0 accumulates src groups in AT_ORDER (expected DMA arrival
    order) — PSUM accumulation is commutative, so the chain starts on
    whichever A^T group lands first.
  - All SBUF tiles live in one pool and both PSUM tag families in another:
    each tile pool costs an all-engine barrier round (~0.5us) at release.
"""

import numpy as np
import ml_dtypes

G, N, F, H, B, L = 4, 1024, 256, 256, 32, 4
N_CORES = 8
NT = N // 128          # 8 node tiles
FT = F // 128          # 2 feature tiles
HT = H // 128          # 2 hidden tiles
NCHUNK = 512           # stage-A moving free dim (one fp32 PSUM bank)
XS = [8.0, 64.0, 256.0, 1024.0]   # fp8 pre-scales for x entering stage A
ASCALE = 1024.0                   # fp8 pre-scale for A^T (entries ~U[0,1]/N)
ZS = [1.0 / 64, 1.0 / 128, 1.0 / 128, 1.0 / 128]  # fp8 pre-scales for zT
GS = 8.0                          # fp8 pre-scale for gcn_w
KWARM = 11                        # PE-warmup dummy matmuls (256 cols each)
IN_ORDER = [0, 2, 1, 3]           # input-layer quarter order (= xt arrival)
C_ORDER = [[0, 1, 2, 3],          # layer-0 stage-A accumulation order per
           [0, 2, 3, 1]]          # dst chunk (= expected A^T half arrivals)

_CACHE = {}


def _split_multi_waits(nc):
    """The walrus build in this container accepts at most ONE sync wait per
    instruction, while Tile's sem-assignment emits up to ~3. Engines execute
    their instruction stream in order, so an instruction's extra waits can be
    hoisted onto same-engine NoOps inserted immediately before it."""
    import concourse.mybir as mybir

    n = 0
    for f in nc.m.functions:
        for bb in f.blocks:
            out = []
            changed = False
            for ins in bb.instructions:
                si = ins.sync_info
                if si is not None and si.on_wait and len(si.on_wait) > 1:
                    waits = list(si.on_wait)
                    for w in waits[:-1]:
                        nop = mybir.InstNoOp(
                            name=f"wsplit_{n}", engine=ins.engine)
                        n += 1
                        nop.sync_info = mybir.SyncInfo(on_wait=[w], on_update=[])
                        out.append(nop)
                    si.on_wait = [waits[-1]]
                    changed = True
                out.append(ins)
            if changed:
                bb.instructions = out
    return nc


def _build_nc(use_bias):
    import concourse.bass as bass
    import concourse.mybir as mybir

    dt = mybir.dt.bfloat16
    d8 = mybir.dt.float8e4
    f32 = mybir.dt.float32
    AF = mybir.ActivationFunctionType

    nc = bass.Bass(enable_partition_id=False, num_swdge_queues=4)
    # DRAM I/O (per core). All inputs are pre-tiled on the host into
    # [128, ...] partition-major contiguous layouts so each DMA moves
    # maximal contiguous runs (strided descriptors measured ~2-4x slower).
    # xt quarter q holds m-tiles {2q, 2q+1} for both k halves:
    #   xt[q][p, k*256 + mc] = X[256q + mc, 128k + p]
    xt_d = nc.dram_tensor("xt", [4, 128, FT * 256], dt, kind="ExternalInput")
    # at[2g+c] = A^T src-group g (256 rows), dst chunk c (512 cols)
    at_d = nc.dram_tensor("at", [8, 128, 2 * NCHUNK], d8, kind="ExternalInput")
    w_in_d = nc.dram_tensor("w_in", [128, FT * H], dt, kind="ExternalInput")
    gw_d = nc.dram_tensor("gw", [128, L * HT * H], d8, kind="ExternalInput")
    r_d = nc.dram_tensor("r", [128, NT * B], dt, kind="ExternalInput")
    if use_bias:
        # biases pre-broadcast over partitions on host: [L+1, 128, H]
        bias_d = nc.dram_tensor("bias", [L + 1, 128, H], f32, kind="ExternalInput")
    out_d = nc.dram_tensor("out", [B, H], f32, kind="ExternalOutput")

    from concourse.tile import TileContext

    with TileContext(nc) as tc:
        import contextlib

        with contextlib.ExitStack() as ctx:
            sb = ctx.enter_context(tc.tile_pool(name="sb", bufs=1))
            ps = ctx.enter_context(tc.tile_pool(name="ps", bufs=4, space="PSUM"))

            # ---- loads: DMAs sized ~128KB (each dma_start costs ~0.6us of
            # issue time on its engine; each ring's FIRST transfer pays
            # ~1.5-2.4us of bootstrap; aggregate inbound is HBM-capped at
            # ~330GB/s). Queues ordered by when each tensor is consumed —
            # A^T ships as 8 (group, chunk) halves so the whole c=0 chunk
            # lands before stage A starts and c=1 streams behind it:
            #   sync:   xt_q0, xt_q1, at[1,0], at[3,0], at[1,1], r
            #   gpsimd: xt_q2, xt_q3, at[2,0], at[2,1], at[3,1], gw23
            #   scalar: w_in,  at[0,0], at[0,1], gw01  (ACT table rides after)
            xt_q = []
            for q, eng in ((0, nc.sync), (1, nc.sync),
                           (2, nc.gpsimd), (3, nc.gpsimd)):
                t = sb.tile([128, FT, 256], dt, tag=f"xtq{q}", name=f"xt_q{q}")
                eng.dma_start(out=t.rearrange("p k m -> p (k m)"), in_=xt_d[q])
                xt_q.append(t)

            wi_b = sb.tile([128, FT, H], dt, tag="wi", name="wi_b")
            nc.scalar.dma_start(out=wi_b.rearrange("p t h -> p (t h)"),
                                in_=w_in_d[:, :])
            w_in = [wi_b[:, k, :] for k in range(FT)]

            at_gc = {}
            at_issue = [((0, 0), nc.scalar), ((1, 0), nc.sync),
                        ((2, 0), nc.gpsimd), ((0, 1), nc.scalar),
                        ((3, 0), nc.sync), ((2, 1), nc.gpsimd),
                        ((1, 1), nc.sync), ((3, 1), nc.gpsimd)]
            for (g, c), eng in at_issue:
                t = sb.tile([128, 2, NCHUNK], d8, tag=f"at{g}{c}",
                            name=f"at_g{g}c{c}")
                eng.dma_start(out=t.rearrange("p t n -> p (t n)"),
                              in_=at_d[2 * g + c])
                at_gc[(g, c)] = t

            # GCN weights in two halves so neither ring's tail blocks layer 0
            gw_h = []
            for i, eng in ((0, nc.scalar), (1, nc.gpsimd)):
                t = sb.tile([128, 2 * HT, H], d8, tag=f"gw{i}", name=f"gw{i}_b")
                eng.dma_start(out=t.rearrange("p t h -> p (t h)"),
                              in_=gw_d[:, 2 * HT * H * i:2 * HT * H * (i + 1)])
                gw_h.append(t)

            r_b = sb.tile([128, NT, B], dt, tag="r", name="r_b")
            nc.sync.dma_start(out=r_b.rearrange("p t b -> p (t b)"), in_=r_d[:, :])
            r = [r_b[:, k, :] for k in range(NT)]
            if use_bias:
                bias = [sb.tile([128, H], f32, tag=f"b{i}", name=f"b{i}") for i in range(L + 1)]
                for i in range(L + 1):
                    nc.sync.dma_start(out=bias[i], in_=bias_d[i])

            # ---- PE warmup: dummy matmuls on memset tiles keep the PE
            # continuously busy through its ~3us DVFS ramp while the input
            # DMAs are in flight. ANY idle gap resets the ramp (measured:
            # post-gap matmuls run at 1.2GHz for another ~3us), so dummies
            # must bridge seamlessly into the first data-gated matmul, and
            # extra fillers pad every spot where a DMA wait could open a gap.
            # DVE does the memsets (no DMA-issue duties, ready earliest).
            warm_l = sb.tile([128, 128], d8, tag="warmL", name="warm_l")
            warm_r = sb.tile([128, 256], d8, tag="warmR", name="warm_r")
            nc.vector.memset(warm_l, 0.0)
            nc.vector.memset(warm_r, 0.0)

            def filler(n):
                for _ in range(n):
                    # psA tag: those banks are idle until layer-0 stage A
                    wps = ps.tile([128, 256], f32, tag="psA", name="warm_ps")
                    nc.tensor.matmul(wps, warm_l, warm_r, start=True, stop=True)

            filler(KWARM)

            # ---- input dense layer: x0 = relu(X @ w_in + b_in) ----
            # fp8 copy (scaled by XS[0]) feeds layer-0 stage A and goes on
            # DVE (needed within ~1us); bf16 copy feeds the residual and is
            # only consumed by the output matmuls ~15us later, so it can sit
            # behind the ACT table load on scalar.
            x0 = [sb.tile([128, 2, H], dt, tag=f"x0_{g}", name=f"x0_{g}")
                  for g in range(NT // 2)]
            x08 = [sb.tile([128, 2, H], d8, tag=f"x8in_{g}", name=f"x08_{g}")
                   for g in range(NT // 2)]
            for gi, g in enumerate(IN_ORDER):
                pst = ps.tile([128, 2, H], f32, tag="psB", name="psB_t")
                for j in range(2):
                    for k in range(FT):
                        xm = xt_q[g][:, k, 128 * j:128 * (j + 1)]
                        nc.tensor.matmul(pst[:, j, :], xm, w_in[k],
                                         start=(k == 0), stop=(k == FT - 1))
                if use_bias:
                    nc.vector.tensor_add(pst[:, 0, :], pst[:, 0, :], bias[0])
                    nc.vector.tensor_add(pst[:, 1, :], pst[:, 1, :], bias[0])
                nc.vector.tensor_scalar(
                    out=x08[g], in0=pst, scalar1=XS[0], scalar2=0.0,
                    op0=mybir.AluOpType.mult, op1=mybir.AluOpType.max)
                nc.scalar.activation(out=x0[g], in_=pst, func=AF.Relu)
            filler(2)           # bridge the input -> stage-A at0 wait

            # ---- GCN layers ----
            NC = N // NCHUNK      # dst chunks per row (2)
            MPC = NCHUNK // 128   # node tiles per chunk (4)
            x_cur = x08
            for layer in range(L):
                # stage A: zT[h, dst] = sum_src x[src, h] * AT[src, dst].
                # c (dst chunk) is the outer loop and each (h, c) gets its own
                # SBUF tile, so stage B's m-tiles in chunk c unblock while
                # stage A still streams chunk c+1 on the PE — no PE bubble
                # waiting on the PSUM->SBUF copies.
                # DoubleRow fp8: out[m,n] = sum_k sum_j lhsT[k,j,m]*rhs[k,j,n]
                # contracts 256 src nodes per matmul (2 fp8 weights per cell).
                # Accumulation over src groups runs in AT_ORDER (arrival
                # order); the two h-chains run the first 3 groups, then their
                # final group, so the in-order PE stream doesn't block on the
                # last-arriving A^T group while ready work could fill the wait.
                zT = [sb.tile([128, HT, NCHUNK], d8, tag=f"zT_{c}",
                              name=f"zT_{layer}_{c}", bufs=2) for c in range(NC)]
                for c in range(NC):
                    order = C_ORDER[c] if layer == 0 else C_ORDER[0]
                    ps_h = [ps.tile([128, NCHUNK], f32, tag="psA",
                                    name="psA_t") for _ in range(HT)]
                    for h in range(HT):
                        for gi in range(3):
                            g = order[gi]
                            nc.tensor.matmul(
                                ps_h[h],
                                x_cur[g][:, :, 128 * h:128 * (h + 1)],
                                at_gc[(g, c)][:, :, :],
                                start=(gi == 0), stop=False,
                                perf_mode=mybir.MatmulPerfMode.DoubleRow)
                    for h in range(HT):
                        g = order[3]
                        nc.tensor.matmul(
                            ps_h[h],
                            x_cur[g][:, :, 128 * h:128 * (h + 1)],
                            at_gc[(g, c)][:, :, :],
                            start=False, stop=True,
                            perf_mode=mybir.MatmulPerfMode.DoubleRow)
                        # scaled fp8 copy; alternate engines so both land in
                        # parallel instead of queueing on DVE
                        if h == 0:
                            nc.scalar.activation(out=zT[c][:, h, :], in_=ps_h[h],
                                                 func=AF.Copy, scale=ZS[layer])
                        else:
                            nc.vector.tensor_scalar_mul(
                                out=zT[c][:, h, :], in0=ps_h[h], scalar1=ZS[layer])
                # stage B: x'[dst, h'] = act(sum_h zT[h, dst] * W[h, h'] + b).
                # gw was pre-divided by 1024*XS[layer] on the host, undoing the
                # fp8 pre-scales. Layers 0..2 write fp8 scaled by XS[layer+1]
                # (next stage A operand); the last layer writes bf16 for the
                # residual add.
                last = layer == L - 1
                if last:
                    x_nxt = [sb.tile([128, 2, H], dt, tag=f"xl_{g}",
                                     name=f"xl_{g}") for g in range(NT // 2)]
                else:
                    x_nxt = [sb.tile([128, 2, H], d8,
                                     tag=f"x8_{layer % 2}_{g}",
                                     name=f"x8_{layer}_{g}")
                             for g in range(NT // 2)]
                # psum here = (ASCALE*XS[l]*ZS[l]*GS) * (z @ W); corr undoes it
                corr = 1.0 / (ASCALE * XS[layer] * ZS[layer] * GS)
                for g in range(NT // 2):
                    pst = ps.tile([128, 2, H], f32, tag="psB", name="psB_t")
                    for j in range(2):
                        m = 2 * g + j
                        c, mc = m // MPC, m % MPC
                        nc.tensor.matmul(
                            pst[:, j, :], zT[c][:, :, 128 * mc:128 * (mc + 1)],
                            gw_h[layer // 2][:, 2 * (layer % 2):2 * (layer % 2) + 2, :],
                            start=True, stop=True,
                            perf_mode=mybir.MatmulPerfMode.DoubleRow)
                    if use_bias:
                        nc.vector.tensor_add(pst[:, 0, :], pst[:, 0, :], bias[layer + 1])
                        nc.vector.tensor_add(pst[:, 1, :], pst[:, 1, :], bias[layer + 1])
                    if last:
                        nc.scalar.activation(out=x_nxt[g], in_=pst, func=AF.Tanh,
                                             scale=corr)
                    elif g % 2 == 0:
                        nc.scalar.activation(out=x_nxt[g], in_=pst,
                                             func=AF.Relu,
                                             scale=XS[layer + 1] * corr)
                    else:
                        nc.vector.tensor_scalar(
                            out=x_nxt[g], in0=pst,
                            scalar1=XS[layer + 1] * corr, scalar2=0.0,
                            op0=mybir.AluOpType.mult, op1=mybir.AluOpType.max)
                x_cur = x_nxt

            # ---- masked-sum matmul; residual folded in:
            # out = R.T@(x4 + x0) = R.T@x4 + R.T@x0 accumulated in one PSUM.
            # Interleave the x0 terms (ready long ago) between the x4 terms
            # (each gated by its tanh) so the PE never idles on ACT. ----
            pso = ps.tile([32, H], f32, tag="psB", name="psB_out")
            for k in range(NT):
                nc.tensor.matmul(pso, r[k][:, :], x0[k // 2][:, k % 2, :],
                                 start=(k == 0), stop=False)
                nc.tensor.matmul(pso, r[k][:, :], x_cur[k // 2][:, k % 2, :],
                                 start=False, stop=(k == NT - 1))
            out_sb = sb.tile([32, H], f32, tag="out", name="out_sb")
            nc.scalar.copy(out=out_sb, in_=pso)
            nc.sync.dma_start(out=out_d[:, :], in_=out_sb)

    _split_multi_waits(nc)
    return nc


def _get_nc(use_bias):
    key = ("nc", use_bias)
    if key not in _CACHE:
        _CACHE[key] = _build_nc(use_bias)
    return _CACHE[key]


def _prepare_in_maps(batch_xs, batch_as, w_in, b_in, gcn_w, gcn_b,
                     graph_idx, cp_mask, use_bias):
    bf16 = ml_dtypes.bfloat16
    fp8 = ml_dtypes.float8_e4m3fn
    mask_f = cp_mask.astype(np.float32)                     # [B, N]

    def ptile(a, inner=128):
        # [T*128, W] -> [128, T*W] partition-major contiguous
        tw = a.reshape(-1, inner, a.shape[-1])
        return np.ascontiguousarray(
            tw.transpose(1, 0, 2).reshape(inner, -1))

    w_in_b = ptile(w_in.astype(bf16))                       # [128, 2*H]
    gw_b = ptile((gcn_w * GS).astype(fp8).reshape(L * H, H))  # [128, 8*H]
    if use_bias:
        # gcn biases add into the scaled stage-B PSUM domain
        bscale = np.array([1.0] + [ASCALE * XS[i] * ZS[i] * GS for i in range(L)],
                          np.float32)
        bias_full = np.concatenate(
            [b_in[None, :], gcn_b], axis=0).astype(np.float32) * bscale[:, None]
        bias_bcast = np.ascontiguousarray(
            np.broadcast_to(bias_full[:, None, :], (L + 1, 128, H)).copy())

    in_maps = []
    for c in range(N_CORES):
        if c < G:
            g = c
            xtf = batch_xs[g].T.astype(bf16)               # [F, N]
            # quarter q: [128, k*256 + mc] = X[256q+mc, 128k+p]
            xt = np.stack([
                ptile(np.ascontiguousarray(xtf[:, 256 * q:256 * (q + 1)]))
                for q in range(4)])                        # [4, 128, 512]
            atf = (batch_as[g].T * ASCALE).astype(fp8)             # [N, N]
            # [8, 128, 2*NCHUNK]: index 2*gg+cc = src group gg, dst chunk cc
            at = np.stack([
                ptile(np.ascontiguousarray(
                    atf[256 * gg:256 * (gg + 1), NCHUNK * cc:NCHUNK * (cc + 1)]))
                for gg in range(4) for cc in range(2)])
            sel = (graph_idx == g).astype(np.float32)[:, None] * mask_f  # [B, N]
            r = ptile(sel.T.astype(bf16))                          # [128, NT*B]
        else:
            xt = np.zeros((4, 128, FT * 256), bf16)
            at = np.zeros((8, 128, 2 * NCHUNK), fp8)
            r = np.zeros((128, NT * B), bf16)
        m = {"xt": xt, "at": at, "w_in": w_in_b, "gw": gw_b, "r": r}
        if use_bias:
            m["bias"] = bias_bcast
        in_maps.append(m)
    return in_maps


def kernel(batch_xs, batch_as, w_in, b_in, gcn_w, gcn_b, graph_idx, cp_mask):
    from concourse import bass_utils

    batch_xs = np.asarray(batch_xs, np.float32)
    batch_as = np.asarray(batch_as, np.float32)
    w_in = np.asarray(w_in, np.float32)
    b_in = np.asarray(b_in, np.float32)
    gcn_w = np.asarray(gcn_w, np.float32)
    gcn_b = np.asarray(gcn_b, np.float32)
    graph_idx = np.asarray(graph_idx).astype(np.int64)
    cp_mask = np.asarray(cp_mask).astype(bool)

    use_bias = bool(np.any(b_in) or np.any(gcn_b))
    nc = _get_nc(use_bias)
    in_maps = _prepare_in_maps(batch_xs, batch_as, w_in, b_in, gcn_w, gcn_b,
                               graph_idx, cp_mask, use_bias)

    res = bass_utils.run_bass_kernel_spmd(nc, in_maps,
                                          core_ids=list(range(N_CORES)))

    partial = np.zeros((B, H), np.float64)
    for c in range(G):
        partial += res.results[c]["out"].astype(np.float64)
    denom = np.maximum(cp_mask.sum(axis=1, keepdims=True).astype(np.float64), 1.0)
    return (partial / denom).astype(np.float32)


# revision 23
# speedup vs baseline: 1.1750x; 1.1750x over previous
"""Trainium2 Bass kernel for the CdfgReader GNN message-passing problem.

Reference computation (shapes hardcoded):
    G, N, F, H, B, L = 4, 1024, 256, 256, 32, 4
    X = batch_xs[graph_idx]          # [B, N, F]
    A = batch_as[graph_idx]          # [B, N, N]
    x = relu(X @ w_in + b_in)
    res = x
    for i in range(L-1): x = relu(A @ x @ gcn_w[i] + gcn_b[i])
    x = tanh(A @ x @ gcn_w[L-1] + gcn_b[L-1])
    x = x + res
    out[b] = masked_mean_over_nodes(x[b], cp_mask[b])   # [B, H]

Key structural insight: the whole forward up to the final masked mean depends
only on which of the G=4 distinct graphs an example selects — so we compute
the forward once per distinct graph (4 graphs) instead of once per example
(32 examples), an 8x FLOP reduction. The per-example masked mean then becomes
a tiny [B,N]x[N,H] matmul against a host-built selection matrix.

Sharding: graph-parallel — core g (g in 0..3) computes graph g's full forward
plus its [B,H] partial of the output; cores 4..7 run the same program on
zeros. The host sums the (disjoint) partials and divides by the mask counts.

Per-core device program (measured max-rel-err ~1.6e-3 end to end — the
output is dominated by the bf16 residual path, so the whole GCN stack runs
in fp8e4m3 with per-tensor pre-scales and DoubleRow matmuls, 2 MACs/cell):
    x0  = relu(XT.T @ w_in)            bf16, 16 matmuls   (lhsT = XT)
    per layer (fp8 DoubleRow):
        zT = (x.T @ AT)                16 matmuls, contraction 256/matmul
        x' = act(zT.T @ W_l)           8 matmuls
    out_partial = R.T@x4 + R.T@x0      bf16, 16 matmuls into one PSUM
The alternating lhsT choice (x -> zT -> x) makes the chain transpose-free;
all activation/copy stages alternate between ACT and DVE so neither gates
the PE stream.

Perf notes (v1):
  - The PE runs at 1.2GHz until it has been continuously busy for ~3us, then
    2.4GHz. KWARM dummy matmuls on memset tiles run during the DMA wait so
    the real matmul stream starts at full clock.
  - xt is split into 4 quarter-descriptors on two DMA rings so the first
    input-layer m-tiles land ~1.5us earlier; each ring's first transfer pays
    ~1.5-2.4us of ring-bootstrap latency, so first-needed tensors go first.
  - Stage-A layer-0 accumulates src groups in AT_ORDER (expected DMA arrival
    order) — PSUM accumulation is commutative, so the chain starts on
    whichever A^T group lands first.
  - All SBUF tiles live in one pool and both PSUM tag families in another:
    each tile pool costs an all-engine barrier round (~0.5us) at release.
"""

import numpy as np
import ml_dtypes

G, N, F, H, B, L = 4, 1024, 256, 256, 32, 4
N_CORES = 8
NT = N // 128          # 8 node tiles
FT = F // 128          # 2 feature tiles
HT = H // 128          # 2 hidden tiles
NCHUNK = 512           # stage-A moving free dim (one fp32 PSUM bank)
XS = [8.0, 64.0, 256.0, 1024.0]   # fp8 pre-scales for x entering stage A
ASCALE = 1024.0                   # fp8 pre-scale for A^T (entries ~U[0,1]/N)
ZS = [1.0 / 64, 1.0 / 128, 1.0 / 128, 1.0 / 128]  # fp8 pre-scales for zT
GS = 8.0                          # fp8 pre-scale for gcn_w
KWARM = 14                        # PE-warmup dummy matmuls (256 cols each)
SX, SW = 4.0, 64.0                # fp8 pre-scales for X and w_in (input MM)
C_ORDER = [[0, 1, 2, 3],          # layer-0 stage-A accumulation order per
           [0, 2, 1, 3]]          # dst chunk (= expected A^T half arrivals)

_CACHE = {}


def _split_multi_waits(nc):
    """The walrus build in this container accepts at most ONE sync wait per
    instruction, while Tile's sem-assignment emits up to ~3. Engines execute
    their instruction stream in order, so an instruction's extra waits can be
    hoisted onto same-engine NoOps inserted immediately before it."""
    import concourse.mybir as mybir

    n = 0
    for f in nc.m.functions:
        for bb in f.blocks:
            out = []
            changed = False
            for ins in bb.instructions:
                si = ins.sync_info
                if si is not None and si.on_wait and len(si.on_wait) > 1:
                    waits = list(si.on_wait)
                    for w in waits[:-1]:
                        nop = mybir.InstNoOp(
                            name=f"wsplit_{n}", engine=ins.engine)
                        n += 1
                        nop.sync_info = mybir.SyncInfo(on_wait=[w], on_update=[])
                        out.append(nop)
                    si.on_wait = [waits[-1]]
                    changed = True
                out.append(ins)
            if changed:
                bb.instructions = out
    return nc


def _build_nc(use_bias):
    import concourse.bass as bass
    import concourse.mybir as mybir

    dt = mybir.dt.bfloat16
    d8 = mybir.dt.float8e4
    f32 = mybir.dt.float32
    AF = mybir.ActivationFunctionType

    nc = bass.Bass(enable_partition_id=False, num_swdge_queues=4)
    # DRAM I/O (per core). All inputs are pre-tiled on the host into
    # [128, ...] partition-major contiguous layouts so each DMA moves
    # maximal contiguous runs (strided descriptors measured ~2-4x slower).
    # xt quarter q holds m-tiles {2q, 2q+1} for both k halves:
    #   xt[q][p, k*256 + mc] = X[256q + mc, 128k + p]
    # xt half hh holds m-tiles 4hh..4hh+3, k-interleaved, scaled by SX in fp8:
    #   xt[hh][p, k*512 + mc] = X[512*hh + mc, 128k + p] * SX
    xt_d = nc.dram_tensor("xt", [2, 128, FT * NCHUNK], d8, kind="ExternalInput")
    # at[2g+c] = A^T src-group g (256 rows), dst chunk c (512 cols)
    at_d = nc.dram_tensor("at", [8, 128, 2 * NCHUNK], d8, kind="ExternalInput")
    w_in_d = nc.dram_tensor("w_in", [128, FT * H], d8, kind="ExternalInput")
    gw_d = nc.dram_tensor("gw", [128, L * HT * H], d8, kind="ExternalInput")
    r_d = nc.dram_tensor("r", [128, NT * B], dt, kind="ExternalInput")
    if use_bias:
        # biases pre-broadcast over partitions on host: [L+1, 128, H]
        bias_d = nc.dram_tensor("bias", [L + 1, 128, H], f32, kind="ExternalInput")
    out_d = nc.dram_tensor("out", [B, H], f32, kind="ExternalOutput")

    from concourse.tile import TileContext

    with TileContext(nc) as tc:
        import contextlib

        with contextlib.ExitStack() as ctx:
            sb = ctx.enter_context(tc.tile_pool(name="sb", bufs=1))
            ps = ctx.enter_context(tc.tile_pool(name="ps", bufs=4, space="PSUM"))

            # ---- loads: DMAs sized ~128KB (each dma_start costs ~0.6us of
            # issue time on its engine; each ring's FIRST transfer pays
            # ~1.5-2.4us of bootstrap; aggregate inbound is HBM-capped at
            # ~330GB/s). Queues ordered by when each tensor is consumed —
            # A^T ships as 8 (group, chunk) halves so the whole c=0 chunk
            # lands before stage A starts and c=1 streams behind it:
            #   sync:   xt_q0, xt_q1, at[1,0], at[3,0], at[1,1], r
            #   gpsimd: xt_q2, xt_q3, at[2,0], at[2,1], at[3,1], gw23
            #   scalar: w_in,  at[0,0], at[0,1], gw01  (ACT table rides after)
            xt_h = []
            for hh, eng in ((0, nc.sync), (1, nc.gpsimd)):
                t = sb.tile([128, FT, NCHUNK], d8, tag=f"xth{hh}",
                            name=f"xt_h{hh}")
                eng.dma_start(out=t.rearrange("p k m -> p (k m)"), in_=xt_d[hh])
                xt_h.append(t)

            wi_b = sb.tile([128, FT, H], d8, tag="wi", name="wi_b")
            nc.scalar.dma_start(out=wi_b.rearrange("p t h -> p (t h)"),
                                in_=w_in_d[:, :])

            at_gc = {}
            at_issue = [((0, 0), nc.scalar), ((1, 0), nc.sync),
                        ((2, 0), nc.gpsimd), ((0, 1), nc.scalar),
                        ((3, 0), nc.sync), ((2, 1), nc.gpsimd),
                        ((1, 1), nc.sync), ((3, 1), nc.gpsimd)]
            for (g, c), eng in at_issue:
                t = sb.tile([128, 2, NCHUNK], d8, tag=f"at{g}{c}",
                            name=f"at_g{g}c{c}")
                eng.dma_start(out=t.rearrange("p t n -> p (t n)"),
                              in_=at_d[2 * g + c])
                at_gc[(g, c)] = t

            # GCN weights in two halves so neither ring's tail blocks layer 0
            gw_h = []
            for i, eng in ((0, nc.scalar), (1, nc.gpsimd)):
                t = sb.tile([128, 2 * HT, H], d8, tag=f"gw{i}", name=f"gw{i}_b")
                eng.dma_start(out=t.rearrange("p t h -> p (t h)"),
                              in_=gw_d[:, 2 * HT * H * i:2 * HT * H * (i + 1)])
                gw_h.append(t)

            r_b = sb.tile([128, NT, B], dt, tag="r", name="r_b")
            nc.sync.dma_start(out=r_b.rearrange("p t b -> p (t b)"), in_=r_d[:, :])
            r = [r_b[:, k, :] for k in range(NT)]
            if use_bias:
                bias = [sb.tile([128, H], f32, tag=f"b{i}", name=f"b{i}") for i in range(L + 1)]
                for i in range(L + 1):
                    nc.sync.dma_start(out=bias[i], in_=bias_d[i])

            # ---- PE warmup: dummy matmuls on memset tiles keep the PE
            # continuously busy through its ~3us DVFS ramp while the input
            # DMAs are in flight. ANY idle gap resets the ramp (measured:
            # post-gap matmuls run at 1.2GHz for another ~3us), so dummies
            # must bridge seamlessly into the first data-gated matmul, and
            # extra fillers pad every spot where a DMA wait could open a gap.
            # DVE does the memsets (no DMA-issue duties, ready earliest).
            warm_l = sb.tile([128, 128], d8, tag="warmL", name="warm_l")
            warm_r = sb.tile([128, 256], d8, tag="warmR", name="warm_r")
            nc.vector.memset(warm_l, 0.0)
            nc.vector.memset(warm_r, 0.0)

            def filler(n):
                for _ in range(n):
                    # psA tag: those banks are idle until layer-0 stage A
                    wps = ps.tile([128, 256], f32, tag="psA", name="warm_ps")
                    nc.tensor.matmul(wps, warm_l, warm_r, start=True, stop=True)

            filler(KWARM)

            # ---- input dense layer: x0 = relu(X @ w_in + b_in) ----
            # fp8 DoubleRow over the k (feature-tile) axis: one matmul per
            # m-tile, contraction 256. PSUM carries SX*SW*(X@w_in) in fp32;
            # the fp8 copy (DVE, scaled to XS[0]) feeds layer-0 stage A, the
            # bf16 residual copy (ACT, scale 1/(SX*SW)) is only consumed by
            # the output matmuls ~15us later so it rides behind the ACT
            # table load.
            incorr = 1.0 / (SX * SW)
            x0 = [sb.tile([128, 2, H], dt, tag=f"x0_{g}", name=f"x0_{g}")
                  for g in range(NT // 2)]
            x08 = [sb.tile([128, 2, H], d8, tag=f"x8in_{g}", name=f"x08_{g}")
                   for g in range(NT // 2)]
            for g in range(NT // 2):
                pst = ps.tile([128, 2, H], f32, tag="psB", name="psB_t")
                for j in range(2):
                    m = 2 * g + j
                    xm = xt_h[m // 4][:, :, 128 * (m % 4):128 * (m % 4 + 1)]
                    nc.tensor.matmul(pst[:, j, :], xm, wi_b,
                                     start=True, stop=True,
                                     perf_mode=mybir.MatmulPerfMode.DoubleRow)
                if use_bias:
                    nc.vector.tensor_add(pst[:, 0, :], pst[:, 0, :], bias[0])
                    nc.vector.tensor_add(pst[:, 1, :], pst[:, 1, :], bias[0])
                nc.vector.tensor_scalar(
                    out=x08[g], in0=pst, scalar1=XS[0] * incorr, scalar2=0.0,
                    op0=mybir.AluOpType.mult, op1=mybir.AluOpType.max)
                nc.scalar.activation(out=x0[g], in_=pst, func=AF.Relu,
                                     scale=incorr)
            filler(2)           # bridge the input -> stage-A at0 wait

            # ---- GCN layers ----
            NC = N // NCHUNK      # dst chunks per row (2)
            MPC = NCHUNK // 128   # node tiles per chunk (4)
            x_cur = x08
            for layer in range(L):
                # stage A: zT[h, dst] = sum_src x[src, h] * AT[src, dst].
                # c (dst chunk) is the outer loop and each (h, c) gets its own
                # SBUF tile, so stage B's m-tiles in chunk c unblock while
                # stage A still streams chunk c+1 on the PE — no PE bubble
                # waiting on the PSUM->SBUF copies.
                # DoubleRow fp8: out[m,n] = sum_k sum_j lhsT[k,j,m]*rhs[k,j,n]
                # contracts 256 src nodes per matmul (2 fp8 weights per cell).
                # Accumulation over src groups runs in AT_ORDER (arrival
                # order); the two h-chains run the first 3 groups, then their
                # final group, so the in-order PE stream doesn't block on the
                # last-arriving A^T group while ready work could fill the wait.
                zT = [sb.tile([128, HT, NCHUNK], d8, tag=f"zT_{c}",
                              name=f"zT_{layer}_{c}", bufs=2) for c in range(NC)]
                for c in range(NC):
                    order = C_ORDER[c] if layer == 0 else C_ORDER[0]
                    ps_h = [ps.tile([128, NCHUNK], f32, tag="psA",
                                    name="psA_t") for _ in range(HT)]
                    for h in range(HT):
                        for gi in range(3):
                            g = order[gi]
                            nc.tensor.matmul(
                                ps_h[h],
                                x_cur[g][:, :, 128 * h:128 * (h + 1)],
                                at_gc[(g, c)][:, :, :],
                                start=(gi == 0), stop=False,
                                perf_mode=mybir.MatmulPerfMode.DoubleRow)
                    for h in range(HT):
                        g = order[3]
                        nc.tensor.matmul(
                            ps_h[h],
                            x_cur[g][:, :, 128 * h:128 * (h + 1)],
                            at_gc[(g, c)][:, :, :],
                            start=False, stop=True,
                            perf_mode=mybir.MatmulPerfMode.DoubleRow)
                        # scaled fp8 copy; alternate engines so both land in
                        # parallel instead of queueing on DVE
                        if h == 0:
                            nc.scalar.activation(out=zT[c][:, h, :], in_=ps_h[h],
                                                 func=AF.Copy, scale=ZS[layer])
                        else:
                            nc.vector.tensor_scalar_mul(
                                out=zT[c][:, h, :], in0=ps_h[h], scalar1=ZS[layer])
                # stage B: x'[dst, h'] = act(sum_h zT[h, dst] * W[h, h'] + b).
                # gw was pre-divided by 1024*XS[layer] on the host, undoing the
                # fp8 pre-scales. Layers 0..2 write fp8 scaled by XS[layer+1]
                # (next stage A operand); the last layer writes bf16 for the
                # residual add.
                last = layer == L - 1
                if last:
                    x_nxt = [sb.tile([128, 2, H], dt, tag=f"xl_{g}",
                                     name=f"xl_{g}") for g in range(NT // 2)]
                else:
                    x_nxt = [sb.tile([128, 2, H], d8,
                                     tag=f"x8_{layer % 2}_{g}",
                                     name=f"x8_{layer}_{g}")
                             for g in range(NT // 2)]
                # psum here = (ASCALE*XS[l]*ZS[l]*GS) * (z @ W); corr undoes it
                corr = 1.0 / (ASCALE * XS[layer] * ZS[layer] * GS)
                for g in range(NT // 2):
                    pst = ps.tile([128, 2, H], f32, tag="psB", name="psB_t")
                    for j in range(2):
                        m = 2 * g + j
                        c, mc = m // MPC, m % MPC
                        nc.tensor.matmul(
                            pst[:, j, :], zT[c][:, :, 128 * mc:128 * (mc + 1)],
                            gw_h[layer // 2][:, 2 * (layer % 2):2 * (layer % 2) + 2, :],
                            start=True, stop=True,
                            perf_mode=mybir.MatmulPerfMode.DoubleRow)
                    if use_bias:
                        nc.vector.tensor_add(pst[:, 0, :], pst[:, 0, :], bias[layer + 1])
                        nc.vector.tensor_add(pst[:, 1, :], pst[:, 1, :], bias[layer + 1])
                    if last:
                        nc.scalar.activation(out=x_nxt[g], in_=pst, func=AF.Tanh,
                                             scale=corr)
                    elif g % 2 == 0:
                        nc.scalar.activation(out=x_nxt[g], in_=pst,
                                             func=AF.Relu,
                                             scale=XS[layer + 1] * corr)
                    else:
                        nc.vector.tensor_scalar(
                            out=x_nxt[g], in0=pst,
                            scalar1=XS[layer + 1] * corr, scalar2=0.0,
                            op0=mybir.AluOpType.mult, op1=mybir.AluOpType.max)
                x_cur = x_nxt

            # ---- masked-sum matmul; residual folded in:
            # out = R.T@(x4 + x0) = R.T@x4 + R.T@x0 accumulated in one PSUM.
            # Interleave the x0 terms (ready long ago) between the x4 terms
            # (each gated by its tanh) so the PE never idles on ACT. ----
            pso = ps.tile([32, H], f32, tag="psB", name="psB_out")
            for k in range(NT):
                nc.tensor.matmul(pso, r[k][:, :], x0[k // 2][:, k % 2, :],
                                 start=(k == 0), stop=False)
                nc.tensor.matmul(pso, r[k][:, :], x_cur[k // 2][:, k % 2, :],
                                 start=False, stop=(k == NT - 1))
            out_sb = sb.tile([32, H], f32, tag="out", name="out_sb")
            nc.scalar.copy(out=out_sb, in_=pso)
            nc.sync.dma_start(out=out_d[:, :], in_=out_sb)

    _split_multi_waits(nc)
    return nc


def _get_nc(use_bias):
    key = ("nc", use_bias)
    if key not in _CACHE:
        _CACHE[key] = _build_nc(use_bias)
    return _CACHE[key]


def _prepare_in_maps(batch_xs, batch_as, w_in, b_in, gcn_w, gcn_b,
                     graph_idx, cp_mask, use_bias):
    bf16 = ml_dtypes.bfloat16
    fp8 = ml_dtypes.float8_e4m3fn
    mask_f = cp_mask.astype(np.float32)                     # [B, N]

    def ptile(a, inner=128):
        # [T*128, W] -> [128, T*W] partition-major contiguous
        tw = a.reshape(-1, inner, a.shape[-1])
        return np.ascontiguousarray(
            tw.transpose(1, 0, 2).reshape(inner, -1))

    w_in_b = ptile((w_in * SW).astype(fp8))                 # [128, 2*H]
    gw_b = ptile((gcn_w * GS).astype(fp8).reshape(L * H, H))  # [128, 8*H]
    if use_bias:
        # biases add into the scaled PSUM domains
        bscale = np.array([SX * SW] + [ASCALE * XS[i] * ZS[i] * GS for i in range(L)],
                          np.float32)
        bias_full = np.concatenate(
            [b_in[None, :], gcn_b], axis=0).astype(np.float32) * bscale[:, None]
        bias_bcast = np.ascontiguousarray(
            np.broadcast_to(bias_full[:, None, :], (L + 1, 128, H)).copy())

    in_maps = []
    for c in range(N_CORES):
        if c < G:
            g = c
            xtf = (batch_xs[g].T * SX).astype(fp8)         # [F, N]
            # half hh: [128, k*512 + mc] = X[512*hh+mc, 128k+p] * SX
            xt = np.stack([
                ptile(np.ascontiguousarray(xtf[:, NCHUNK * hh:NCHUNK * (hh + 1)]))
                for hh in range(2)])                       # [2, 128, 1024]
            atf = (batch_as[g].T * ASCALE).astype(fp8)             # [N, N]
            # [8, 128, 2*NCHUNK]: index 2*gg+cc = src group gg, dst chunk cc
            at = np.stack([
                ptile(np.ascontiguousarray(
                    atf[256 * gg:256 * (gg + 1), NCHUNK * cc:NCHUNK * (cc + 1)]))
                for gg in range(4) for cc in range(2)])
            sel = (graph_idx == g).astype(np.float32)[:, None] * mask_f  # [B, N]
            r = ptile(sel.T.astype(bf16))                          # [128, NT*B]
        else:
            xt = np.zeros((2, 128, FT * NCHUNK), fp8)
            at = np.zeros((8, 128, 2 * NCHUNK), fp8)
            r = np.zeros((128, NT * B), bf16)
        m = {"xt": xt, "at": at, "w_in": w_in_b, "gw": gw_b, "r": r}
        if use_bias:
            m["bias"] = bias_bcast
        in_maps.append(m)
    return in_maps


def kernel(batch_xs, batch_as, w_in, b_in, gcn_w, gcn_b, graph_idx, cp_mask):
    from concourse import bass_utils

    batch_xs = np.asarray(batch_xs, np.float32)
    batch_as = np.asarray(batch_as, np.float32)
    w_in = np.asarray(w_in, np.float32)
    b_in = np.asarray(b_in, np.float32)
    gcn_w = np.asarray(gcn_w, np.float32)
    gcn_b = np.asarray(gcn_b, np.float32)
    graph_idx = np.asarray(graph_idx).astype(np.int64)
    cp_mask = np.asarray(cp_mask).astype(bool)

    use_bias = bool(np.any(b_in) or np.any(gcn_b))
    nc = _get_nc(use_bias)
    in_maps = _prepare_in_maps(batch_xs, batch_as, w_in, b_in, gcn_w, gcn_b,
                               graph_idx, cp_mask, use_bias)

    res = bass_utils.run_bass_kernel_spmd(nc, in_maps,
                                          core_ids=list(range(N_CORES)))

    partial = np.zeros((B, H), np.float64)
    for c in range(G):
        partial += res.results[c]["out"].astype(np.float64)
    denom = np.maximum(cp_mask.sum(axis=1, keepdims=True).astype(np.float64), 1.0)
    return (partial / denom).astype(np.float32)
